# revision 1
# baseline (speedup 1.0000x reference)
"""Trainium2 Bass kernel for LinearAttention4 (self-contained, v2: bf16).

Problem (per sample): x [256, 56, 56] fp32
  qk = elu(conv1x1(x; qk_w, qk_b)) + 1 ; q, k = split(qk)
  kv = k @ v.T / n ; num = q.T @ kv ; den = q.T @ mean(k) + 1e-6
  attn = (num / den).T ; out = attn + depthwise3x3(x; pe_w) + pe_b

Sharding: data-parallel over batch, 4 samples per core on 8 NeuronCores.

v2 design (vs v1 f32r baseline, 292us/core in the cost model):
- Everything on-chip is bf16 (PSUM accumulation stays f32): halves input
  DMA bytes and enables bf16 matmul rates (1 cyc/row at any free size).
- Spatial layout is padded 58x58 -> 3364, tail-padded to 3456 = 27*128.
- Host sends BOTH x (padded, channel-major) and xT (transposed, position-
  major, 257 cols = 256 channels + an interior-masked ones column). The
  kv matmul over 27 position-chunks then yields ksum for free in col 256,
  with pad positions masked by zero rows of xT -- no memsets of k needed.
- k is transposed on-chip by ONE XBAR DMA-transpose instruction per
  sample ([128,3456] -> 27x[128,128] tiles) -- no PE transposes, no psum
  evacuation copies.
- elu(z)+1 = min(exp(z),1) + relu(z): ACT does exp and relu, DVE/Pool
  combine. Full 464-col spans are written (pad cols get elu(bias)+1,
  finite; they are masked by xT's zero rows / skipped at evacuation).
- depthwise 3x3: 9 taps per (cb, chunk). Most chunks run all 9 taps as
  diagonal bf16 matmuls accumulating into the num psum tile (193ns each
  at ramped PE clock); a tunable subset runs as vector MAC chains
  (tensor_scalar init + scalar_tensor_tensor MACs) on DVE/Pool into a
  bf16 SBUF accumulator, combined at evacuation. This balances PE vs
  vector engines.
- den chunks are evacuated psum->SBUF by tiny DMAs; reciprocal is
  computed folded [116,29]; the recip row is broadcast to 128 partitions
  by a rank-1 ones-matmul; q is scaled in place (exact: the scale
  commutes past the num matmul).
"""

import numpy as np
import ml_dtypes

import concourse.bass as bass
import concourse.mybir as mybir
from concourse.tile import TileContext
from concourse.bass_utils import run_bass_kernel_spmd

F32 = mybir.dt.float32
BF16 = mybir.dt.bfloat16

B, C, H, W = 32, 256, 56, 56
N = H * W  # 3136
NCORES = 8
SPC = B // NCORES  # 4
HP = H + 2  # 58
NP = HP * HP  # 3364
JT = 27  # transpose / xT chunks of 128 positions
NPP = JT * 128  # 3456 padded positions incl. tail zeros
SPAN = 8 * HP  # 464 cols per span chunk (8 padded rows)
NCH = 7  # span chunks of 8 interior rows
EPS = 1e-6 * N  # den eps, rescaled because kv/ksum stay unscaled

# which (cb, ch) chunks run their 9 conv taps on vector engines instead of
# PE diag-matmuls; tune for engine balance
VEC_CHUNKS = set()
# engine per MAC index 0..7 within a vector chunk: 'd' = DVE, 'p' = Pool
VEC_MAC_ENG = "ppdppdpp"


def _hoist_waits(nc, kinds, max_waits):
    """Walrus allows at most one SyncWait on most instructions and none on
    DMA-transpose; hoist extras onto fresh same-engine NoOps placed
    immediately before (same semantics: in-order queues)."""
    for f in nc.m.functions:
        for blk in f.blocks:
            new_insts = []
            for ins in blk.instructions:
                si = ins.sync_info
                waits = list(si.on_wait) if si is not None else []
                if (kinds is None or isinstance(ins, kinds)) and len(waits) > max_waits:
                    head = waits if max_waits == 0 else waits[:-max_waits]
                    tail = [] if max_waits == 0 else waits[-max_waits:]
                    for w in head:
                        nop = mybir.InstNoOp(
                            name=f"Wsplit-{nc.next_id()}", engine=ins.engine,
                            ins=[], outs=[],
                        )
                        nop.sync_info = mybir.SyncInfo(on_wait=[w], on_update=[])
                        new_insts.append(nop)
                    ins.sync_info = mybir.SyncInfo(
                        on_wait=tail, on_update=list(si.on_update)
                    )
                new_insts.append(ins)
            blk.instructions = new_insts


def _build():
    nc = bass.Bass()
    # all DRAM params flat 1D: PJRT/XLA may permute multi-dim layouts
    xs_f = nc.declare_dram_parameter("xs", [SPC * 2 * 128 * NPP], BF16, isOutput=False)
    xts_f = nc.declare_dram_parameter("xts", [SPC * JT * 128 * 257], BF16, isOutput=False)
    wqkT_f = nc.declare_dram_parameter("wqkT", [2 * 128 * 256], BF16, isOutput=False)
    wtap_f = nc.declare_dram_parameter("wtap", [2 * 9 * 128 * 128], BF16, isOutput=False)
    wtapv_f = nc.declare_dram_parameter("wtapv", [128 * 18], F32, isOutput=False)
    onesb_f = nc.declare_dram_parameter("onesb", [128], BF16, isOutput=False)
    biasqk_f = nc.declare_dram_parameter("biasqk", [128 * 2], F32, isOutput=False)
    peb_f = nc.declare_dram_parameter("peb", [128 * 2], F32, isOutput=False)
    out_f = nc.declare_dram_parameter("out", [SPC * 2 * 128 * N], F32, isOutput=True)
    xs = xs_f[:].rearrange("(s c p n) -> s c p n", s=SPC, c=2, p=128)
    xts = xts_f[:].rearrange("(s j p n) -> s j p n", s=SPC, j=JT, p=128)
    out = out_f[:].rearrange("(s c p n) -> s c p n", s=SPC, c=2, p=128)

    Exp = mybir.ActivationFunctionType.Exp
    Relu = mybir.ActivationFunctionType.Relu
    mi, ad, mx, mu = (mybir.AluOpType.min, mybir.AluOpType.add,
                      mybir.AluOpType.max, mybir.AluOpType.mult)

    def span_start(ch):
        return HP * (1 + 8 * ch) + 1

    with TileContext(nc) as tc:
        with (
            tc.tile_pool(name="wp", bufs=1) as wp,
            tc.tile_pool(name="xpool", bufs=2) as xpool,
            tc.tile_pool(name="xtpool", bufs=2) as xtpool,
            tc.tile_pool(name="qkpool", bufs=2) as qkpool,
            tc.tile_pool(name="ktpool", bufs=2) as ktpool,
            tc.tile_pool(name="erpool", bufs=3) as erpool,
            tc.tile_pool(name="kvpool", bufs=2) as kvpool,
            tc.tile_pool(name="denpool", bufs=2) as denpool,
            tc.tile_pool(name="accpool", bufs=3) as accpool,
            tc.tile_pool(name="opool", bufs=2) as opool,
            tc.tile_pool(name="qkps", bufs=2, space="PSUM") as qkps,
            tc.tile_pool(name="kvps", bufs=1, space="PSUM") as kvps,
            tc.tile_pool(name="dbps", bufs=2, space="PSUM") as dbps,
            tc.tile_pool(name="bcps", bufs=1, space="PSUM") as bcps,
            tc.tile_pool(name="numps", bufs=2, space="PSUM") as numps,
        ):
            w_qk = wp.tile([128, 512], BF16, name="w_qk")
            w_tap = wp.tile([128, 2304], BF16, name="w_tap")
            w_tapv = wp.tile([128, 18], F32, name="w_tapv")
            w_ones = wp.tile([1, 128], BF16, name="w_ones")
            w_bqk = wp.tile([128, 2], F32, name="w_bqk")
            w_peb = wp.tile([128, 2], F32, name="w_peb")
            dma = nc.default_dma_engine.dma_start
            dma(
                out=w_qk[:].rearrange("p (c o) -> p c o", c=2),
                in_=wqkT_f[:].rearrange("(c p o) -> p c o", c=2, p=128),
            )
            dma(
                out=w_tap[:].rearrange("p (c t j) -> p c t j", c=2, t=9),
                in_=wtap_f[:].rearrange("(c t p j) -> p c t j", c=2, t=9, p=128),
            )
            dma(out=w_tapv[:], in_=wtapv_f[:].rearrange("(p j) -> p j", p=128))
            dma(out=w_ones[:], in_=onesb_f[:].rearrange("(a j) -> a j", a=1))
            dma(out=w_bqk[:], in_=biasqk_f[:].rearrange("(p c) -> p c", p=128))
            dma(out=w_peb[:], in_=peb_f[:].rearrange("(p c) -> p c", p=128))

            def front_a(s, st):
                # ---- A: load x (padded chan-major) + xT (pos-major) -------
                xp = xpool.tile([128, 2 * NPP], BF16, tag="xp", name="xp")
                dma(
                    out=xp[:].rearrange("p (c n) -> p c n", c=2),
                    in_=xs[s].rearrange("c p n -> p c n"),
                )
                xt = xtpool.tile([128, JT * 257], BF16, tag="xt", name="xt")
                dma(
                    out=xt[:].rearrange("p (j n) -> p j n", j=JT),
                    in_=xts[s].rearrange("j p n -> p j n"),
                )
                st.update(xp=xp, xt=xt)

                # ---- B: qk matmul + elu+1 (full spans; k first) -----------
                q_elu = qkpool.tile([128, NPP], BF16, tag="qelu", name="q_elu")
                k_elu = qkpool.tile([128, NPP], BF16, tag="kelu", name="k_elu")
                nc.vector.memset(k_elu[:, 0:59], 0)
                nc.vector.memset(k_elu[:, HP * 57 + 1 : NPP], 0)
                st.update(q=q_elu, k=k_elu)
                for mb in (1, 0):  # k first: unblocks transpose + kv early
                    dst = q_elu if mb == 0 else k_elu
                    for ch in range(NCH):
                        p1 = span_start(ch)
                        ps = qkps.tile([128, SPAN], F32, tag="qkps", name="ps")
                        for cc in range(2):
                            nc.tensor.matmul(
                                ps[:],
                                w_qk[:, 256 * cc + 128 * mb : 256 * cc + 128 * mb + 128],
                                xp[:, NPP * cc + p1 : NPP * cc + p1 + SPAN],
                                start=(cc == 0),
                                stop=(cc == 1),
                            )
                        e = erpool.tile([128, SPAN], BF16, tag="e", name="e")
                        r = erpool.tile([128, SPAN], BF16, tag="r", name="r")
                        nc.scalar.activation(
                            e[:], ps[:], Exp, bias=w_bqk[:, mb : mb + 1], scale=1.0
                        )
                        nc.scalar.activation(
                            r[:], ps[:], Relu, bias=w_bqk[:, mb : mb + 1], scale=1.0
                        )
                        nc.vector.scalar_tensor_tensor(
                            dst[:, p1 : p1 + SPAN], e[:], 1.0, r[:], op0=mi, op1=ad
                        )
                    if mb == 1:
                        # ---- C: k transpose via XBAR DMA ------------------
                        kT = ktpool.tile([128, JT * 128], BF16, tag="kt", name="kT")
                        nc.default_dma_engine.dma_start_transpose(
                            kT[:].rearrange("p (j c) -> p j c", j=JT), k_elu[:]
                        )
                        st.update(kT=kT)

            def front_b(s, st):
                xp, xt, q_elu, kT = st["xp"], st["xt"], st["q"], st["kT"]
                # ---- D: kv (+ksum in col 256) -----------------------------
                kvp = kvps.tile([128, 257], F32, tag="kvps", name="kvp")
                for j in range(JT):
                    nc.tensor.matmul(
                        kvp[:], kT[:, 128 * j : 128 * (j + 1)],
                        xt[:, 257 * j : 257 * (j + 1)],
                        start=(j == 0), stop=(j == JT - 1),
                    )
                kv_sb = kvpool.tile([128, 257], BF16, tag="kv", name="kv_sb")
                nc.vector.tensor_copy(kv_sb[:], kvp[:])
                st.update(kv=kv_sb)

                # ---- E: den + reciprocal ----------------------------------
                den = denpool.tile([1, NP], F32, tag="den", name="den")
                nc.vector.memset(den[:, 0:59], 1.0)
                nc.vector.memset(den[:, HP * 57 + 1 : NP], 1.0)
                for ch in range(NCH):
                    p1 = span_start(ch)
                    dp = dbps.tile([1, SPAN], F32, tag="dbps", name="dp")
                    nc.tensor.matmul(
                        dp[:], kv_sb[:, 256:257],
                        q_elu[:, p1 : p1 + SPAN],
                        start=True, stop=True,
                    )
                    nc.vector.tensor_copy(den[:, p1 : p1 + SPAN], dp[:])
                recf = denpool.tile([116, 29], F32, tag="recf", name="recf")
                dma(out=recf[:], in_=den[:])
                nc.vector.tensor_scalar_add(recf[:], recf[:], EPS)
                nc.vector.reciprocal(recf[:], recf[:])
                recb = denpool.tile([116, 29], BF16, tag="recb", name="recb")
                with nc.allow_low_precision(reason="bf16 recip row: 4e-3 rel ok"):
                    nc.vector.tensor_copy(recb[:], recf[:])
                rrow = denpool.tile([1, NP], BF16, tag="rrow", name="rrow")
                dma(out=rrow[:], in_=recb[:])
                st.update(rrow=rrow)

            def back(s, st):
                xp, q_elu, kv_sb, rrow = st["xp"], st["q"], st["kv"], st["rrow"]
                # ---- F1: recip broadcast + q scaling ----------------------
                for ch in range(NCH):
                    p1 = span_start(ch)
                    bc = bcps.tile([128, SPAN], F32, tag="bcps", name="bc")
                    nc.tensor.matmul(
                        bc[:], w_ones[:], rrow[:, p1 : p1 + SPAN],
                        start=True, stop=True,
                    )
                    nc.vector.tensor_tensor(
                        q_elu[:, p1 : p1 + SPAN], q_elu[:, p1 : p1 + SPAN],
                        bc[:], op=mu,
                    )

                # ---- F2: vector conv chains for VEC chunks ----------------
                accs = {}
                for cb in range(2):
                    for ch in range(NCH):
                        if (cb, ch) not in VEC_CHUNKS:
                            continue
                        p1 = span_start(ch)
                        acc = accpool.tile([128, SPAN], BF16, tag="acc", name="acc")
                        accs[(cb, ch)] = acc
                        base = NPP * cb + p1
                        nc.vector.tensor_scalar(
                            out=acc[:], in0=xp[:, base : base + SPAN],
                            scalar1=w_tapv[:, 9 * cb + 4 : 9 * cb + 5],
                            scalar2=None, op0=mu,
                        )
                        for t in range(9):
                            if t == 4:
                                continue
                            ky, kx = t // 3, t % 3
                            off = HP * (ky - 1) + (kx - 1)
                            nc.vector.scalar_tensor_tensor(
                                acc[:], xp[:, base + off : base + off + SPAN],
                                w_tapv[:, 9 * cb + t : 9 * cb + t + 1], acc[:],
                                op0=mu, op1=ad,
                            )

                # ---- F3: num (+PE taps) + evacuation ----------------------
                for cb in range(2):
                    o_sb = opool.tile([128, N], F32, tag="osb", name="o_sb")
                    for ch in range(NCH):
                        p1 = span_start(ch)
                        is_vec = (cb, ch) in VEC_CHUNKS
                        pn = numps.tile([128, SPAN], F32, tag="numps", name="pn")
                        nc.tensor.matmul(
                            pn[:], kv_sb[:, 128 * cb : 128 * (cb + 1)],
                            q_elu[:, p1 : p1 + SPAN],
                            start=True, stop=is_vec,
                        )
                        if not is_vec:
                            for t in range(9):
                                ky, kx = t // 3, t % 3
                                off = HP * (ky - 1) + (kx - 1)
                                nc.tensor.matmul(
                                    pn[:],
                                    w_tap[:, 1152 * cb + 128 * t : 1152 * cb + 128 * (t + 1)],
                                    xp[:, NPP * cb + p1 + off : NPP * cb + p1 + off + SPAN],
                                    start=False, stop=(t == 8),
                                )
                        o_v = o_sb[:, 448 * ch : 448 * (ch + 1)].rearrange(
                            "p (y x) -> p y x", x=56
                        )
                        pn_v = pn[:].rearrange("p (y x) -> p y x", x=HP)[:, :, 0:56]
                        if is_vec:
                            acc = accs[(cb, ch)]
                            acc_v = acc[:].rearrange("p (y x) -> p y x", x=HP)[:, :, 0:56]
                            nc.vector.scalar_tensor_tensor(
                                o_v, pn_v, w_peb[:, cb : cb + 1], acc_v,
                                op0=ad, op1=ad,
                            )
                        else:
                            nc.scalar.activation(
                                o_v, pn_v, mybir.ActivationFunctionType.Identity,
                                bias=w_peb[:, cb : cb + 1], scale=1.0,
                            )
                    dma(out=out[s, cb], in_=o_sb[:])

            states = [dict() for _ in range(SPC)]
            for s in range(SPC):
                front_a(s, states[s])
                if s >= 1:
                    back(s - 1, states[s - 1])
                front_b(s, states[s])
            back(SPC - 1, states[SPC - 1])

    _hoist_waits(nc, (mybir.InstDmaTransposeAnt,), 0)
    _hoist_waits(nc, None, 1)
    return nc


_NC_CACHE = []


def _host_prep(x, qk_w, qk_b, pe_w, pe_b):
    bf16 = ml_dtypes.bfloat16
    # padded 58x58 spatial layout, tail-padded to 3456; c in two 128-blocks
    xp = np.zeros((B, 2, 128, HP, HP), np.float32)
    xp[:, :, :, 1 : H + 1, 1 : W + 1] = x.reshape(B, 2, 128, H, W)
    xpf = np.zeros((B, 2, 128, NPP), np.float32)
    xpf[:, :, :, :NP] = xp.reshape(B, 2, 128, NP)
    xs = xpf.astype(bf16)

    # transposed layout with interior-masked ones column
    xT = xpf.reshape(B, 256, NPP).transpose(0, 2, 1)  # [B, NPP, 256]
    pos = np.arange(NPP)
    interior = ((pos < NP) & ((pos // HP) >= 1) & ((pos // HP) <= H)
                & ((pos % HP) >= 1) & ((pos % HP) <= W))
    xts = np.concatenate(
        [xT, np.broadcast_to(interior[None, :, None], (B, NPP, 1)).astype(np.float32)],
        axis=2,
    )  # [B, NPP, 257]
    xts = np.ascontiguousarray(xts.reshape(B, JT, 128, 257)).astype(bf16)

    wqkT = np.ascontiguousarray(qk_w.T).reshape(2, 128, 256).astype(bf16)
    wtap = np.zeros((2, 9, 128, 128), np.float32)
    idx = np.arange(128)
    for cb in range(2):
        for t in range(9):
            wtap[cb, t, idx, idx] = pe_w[128 * cb : 128 * (cb + 1), 0, t // 3, t % 3]
    wtap = wtap.astype(bf16)
    # per-partition tap scalars for vector chains (scalar APs must be f32)
    wtapv = np.zeros((128, 18), np.float32)
    for cb in range(2):
        for t in range(9):
            wtapv[:, 9 * cb + t] = pe_w[128 * cb : 128 * (cb + 1), 0, t // 3, t % 3]
    biasqk = np.stack([qk_b[:128], qk_b[128:]], axis=1).copy()  # [128, 2]
    pebh = np.stack([pe_b[:128], pe_b[128:]], axis=1).copy()

    shared = {
        "wqkT": wqkT.ravel(),
        "wtap": wtap.ravel(),
        "wtapv": wtapv.ravel(),
        "onesb": np.ones(128, bf16),
        "biasqk": biasqk.ravel(),
        "peb": pebh.ravel(),
    }
    return xs, xts, shared


def kernel(x, qk_w, qk_b, pe_w, pe_b):
    x = np.asarray(x, np.float32)
    qk_w = np.asarray(qk_w, np.float32)
    qk_b = np.asarray(qk_b, np.float32)
    pe_w = np.asarray(pe_w, np.float32)
    pe_b = np.asarray(pe_b, np.float32)

    xs, xts, shared = _host_prep(x, qk_w, qk_b, pe_w, pe_b)
    in_maps = [
        {
            "xs": np.ascontiguousarray(xs[r * SPC : (r + 1) * SPC]).ravel(),
            "xts": np.ascontiguousarray(xts[r * SPC : (r + 1) * SPC]).ravel(),
            **shared,
        }
        for r in range(NCORES)
    ]

    if not _NC_CACHE:
        _NC_CACHE.append(_build())
    nc = _NC_CACHE[0]
    res = run_bass_kernel_spmd(nc, in_maps, list(range(NCORES)))

    full = np.empty((B, C, H, W), np.float32)
    for r in range(NCORES):
        o = res.results[r]["out"].reshape(SPC, 2, 128, N)
        full[r * SPC : (r + 1) * SPC] = o.reshape(SPC, C, H, W)
    return full



# revision 6
# speedup vs baseline: 1.1625x; 1.1625x over previous
"""Trainium2 Bass kernel for LinearAttention4 (self-contained, v3: fp8 DR).

Problem (per sample): x [256, 56, 56] fp32
  qk = elu(conv1x1(x; qk_w, qk_b)) + 1 ; q, k = split(qk)
  kv = k @ v.T / n ; num = q.T @ kv ; den = q.T @ mean(k) + 1e-6
  attn = (num / den).T ; out = attn + depthwise3x3(x; pe_w) + pe_b

Sharding: data-parallel over batch, 4 samples per core on 8 NeuronCores.

v3 design (vs v2 bf16 baseline, 239us in the cost model):
- fp8e4m3 everywhere the contraction is long enough to wash quantization
  noise (empirically verified): qk conv, k^T production, kv outer product.
  DoubleRow perf mode pairs two 128-deep k-tiles per matmul at 0.5 cyc/col
  (4x bf16): qk contracts its two 128-channel blocks in one instruction;
  kv contracts position-chunk pairs.
- k is produced directly POSITION-MAJOR (kT) by DR matmuls (lhsT = x
  chunk, rhs = w_k), killing the XBAR DMA-transpose and yielding fp8 kT
  for the kv DR matmuls. Per-channel k bias lives in the free dim there,
  so it is accumulated into the psum group by a ones[1,128] x bias_row
  matmul before the DR matmuls.
- Depthwise taps stay bf16 diag matmuls (fp8 taps measured 6.8e-2 rel err
  vs the 2e-2 gate: 9-term products do not average the noise out); a
  tunable subset of (cb, ch) chunks runs on DVE/Pool MAC chains instead.
- den reciprocal via ACT Reciprocal straight from den psum (eps dropped:
  den >= 128*exp(-20)*ksum_min, safely positive).
- elu combine decomposed into 4x/2x-capable DVE ops: relu = ts(max,0),
  min = ts(min,1) [4x], add = tt [2x] instead of 1x stt.
- Evacuation on the (otherwise idle) Pool engine: tensor_scalar add-peb,
  psum -> fp16 SBUF; output DMA'd as fp16 and upcast on host.
"""

import numpy as np
import ml_dtypes

import concourse.bass as bass
import concourse.mybir as mybir
from concourse.tile import TileContext
from concourse.bass_utils import run_bass_kernel_spmd

F32 = mybir.dt.float32
BF16 = mybir.dt.bfloat16
F16 = mybir.dt.float16
FP8 = mybir.dt.float8e4
DR = mybir.MatmulPerfMode.DoubleRow

B, C, H, W = 32, 256, 56, 56
N = H * W  # 3136
NCORES = 8
SPC = B // NCORES  # 4
HP = H + 2  # 58
NP = HP * HP  # 3364
JT = 27  # position chunks of 128
NPP = JT * 128  # 3456 padded positions incl. tail zeros
SPAN = 8 * HP  # 464 cols per span chunk (8 interior rows)
NCH = 7  # span chunks of 8 interior rows

# kT psum grouping: 27 chunks -> groups of 4,4,4,4,4,4,3
KT_GROUPS = [list(range(4 * g, min(4 * g + 4, JT))) for g in range(7)]

# tap-chunk offload: (cb, ch) sets running MAC chains on DVE / Pool
VEC_DVE = set()
VEC_POOL = set()
# engine for the q-path relu: "dve" or "act"
RELU_Q = "dve"


def _hoist_waits(nc, kinds, max_waits):
    """Walrus allows at most one SyncWait on most instructions; hoist extras
    onto fresh same-engine NoOps placed immediately before (same semantics:
    in-order queues)."""
    for f in nc.m.functions:
        for blk in f.blocks:
            new_insts = []
            for ins in blk.instructions:
                si = ins.sync_info
                waits = list(si.on_wait) if si is not None else []
                if (kinds is None or isinstance(ins, kinds)) and len(waits) > max_waits:
                    head = waits if max_waits == 0 else waits[:-max_waits]
                    tail = [] if max_waits == 0 else waits[-max_waits:]
                    for w in head:
                        nop = mybir.InstNoOp(
                            name=f"Wsplit-{nc.next_id()}", engine=ins.engine,
                            ins=[], outs=[],
                        )
                        nop.sync_info = mybir.SyncInfo(on_wait=[w], on_update=[])
                        new_insts.append(nop)
                    ins.sync_info = mybir.SyncInfo(
                        on_wait=tail, on_update=list(si.on_update)
                    )
                new_insts.append(ins)
            blk.instructions = new_insts


def _build():
    nc = bass.Bass()
    # all DRAM params flat 1D: PJRT/XLA may permute multi-dim layouts
    xs8_f = nc.declare_dram_parameter("xs8", [SPC * 2 * 128 * NPP], FP8, isOutput=False)
    xsb_f = nc.declare_dram_parameter("xsb", [SPC * 2 * 128 * NPP], BF16, isOutput=False)
    xts_f = nc.declare_dram_parameter("xts", [SPC * JT * 128 * 257], FP8, isOutput=False)
    wqk8_f = nc.declare_dram_parameter("wqk8", [128 * 512], FP8, isOutput=False)
    wkb_f = nc.declare_dram_parameter("wkb", [512], BF16, isOutput=False)
    wtap_f = nc.declare_dram_parameter("wtap", [2 * 9 * 128 * 128], BF16, isOutput=False)
    wtapv_f = nc.declare_dram_parameter("wtapv", [128 * 18], F32, isOutput=False)
    onesb_f = nc.declare_dram_parameter("onesb", [128], BF16, isOutput=False)
    biasq_f = nc.declare_dram_parameter("biasq", [128], F32, isOutput=False)
    peb_f = nc.declare_dram_parameter("peb", [128 * 2], F32, isOutput=False)
    out_f = nc.declare_dram_parameter("out", [SPC * 2 * 128 * N], F16, isOutput=True)
    xs8 = xs8_f[:].rearrange("(s c p n) -> s c p n", s=SPC, c=2, p=128)
    xsb = xsb_f[:].rearrange("(s c p n) -> s c p n", s=SPC, c=2, p=128)
    xts = xts_f[:].rearrange("(s j p n) -> s j p n", s=SPC, j=JT, p=128)
    out = out_f[:].rearrange("(s c p n) -> s c p n", s=SPC, c=2, p=128)

    Exp = mybir.ActivationFunctionType.Exp
    Recip = mybir.ActivationFunctionType.Reciprocal
    Relu = mybir.ActivationFunctionType.Relu
    mi, ad, mx, mu = (mybir.AluOpType.min, mybir.AluOpType.add,
                      mybir.AluOpType.max, mybir.AluOpType.mult)

    def span_start(ch):
        return HP * (1 + 8 * ch) + 1

    with TileContext(nc) as tc:
        with (
            tc.tile_pool(name="wp", bufs=1) as wp,
            tc.tile_pool(name="xpool", bufs=2) as xpool,
            tc.tile_pool(name="xbpool", bufs=2) as xbpool,
            tc.tile_pool(name="xtpool", bufs=2) as xtpool,
            tc.tile_pool(name="qpool", bufs=2) as qpool,
            tc.tile_pool(name="ktpool", bufs=2) as ktpool,
            tc.tile_pool(name="erpool", bufs=4) as erpool,
            tc.tile_pool(name="er2pool", bufs=4) as er2pool,
            tc.tile_pool(name="kvpool", bufs=2) as kvpool,
            tc.tile_pool(name="denpool", bufs=2) as denpool,
            tc.tile_pool(name="accpool", bufs=3) as accpool,
            tc.tile_pool(name="opool", bufs=2) as opool,
            tc.tile_pool(name="qps", bufs=1, space="PSUM") as qps,
            tc.tile_pool(name="ktps", bufs=2, space="PSUM") as ktps,
            tc.tile_pool(name="kvps", bufs=1, space="PSUM") as kvps,
            tc.tile_pool(name="dbps", bufs=1, space="PSUM") as dbps,
            tc.tile_pool(name="bcps", bufs=1, space="PSUM") as bcps,
            tc.tile_pool(name="numps", bufs=2, space="PSUM") as numps,
        ):
            w_qk8 = wp.tile([128, 512], FP8, name="w_qk8")
            w_kb = wp.tile([1, 512], BF16, name="w_kb")
            w_tap = wp.tile([128, 2304], BF16, name="w_tap")
            w_tapv = wp.tile([128, 18], F32, name="w_tapv")
            w_ones = wp.tile([1, 128], BF16, name="w_ones")
            w_bq = wp.tile([128, 1], F32, name="w_bq")
            w_peb = wp.tile([128, 2], F32, name="w_peb")
            dma = nc.default_dma_engine.dma_start
            dma(out=w_qk8[:], in_=wqk8_f[:].rearrange("(p k) -> p k", p=128))
            dma(out=w_kb[:], in_=wkb_f[:].rearrange("(a k) -> a k", a=1))
            dma(
                out=w_tap[:].rearrange("p (c t j) -> p c t j", c=2, t=9),
                in_=wtap_f[:].rearrange("(c t p j) -> p c t j", c=2, t=9, p=128),
            )
            dma(out=w_tapv[:], in_=wtapv_f[:].rearrange("(p j) -> p j", p=128))
            dma(out=w_ones[:], in_=onesb_f[:].rearrange("(a j) -> a j", a=1))
            dma(out=w_bq[:], in_=biasq_f[:].rearrange("(p a) -> p a", p=128))
            dma(out=w_peb[:], in_=peb_f[:].rearrange("(p c) -> p c", p=128))
            # [128, 2, 128] DR views of the packed qk weight: (cc, mb, o)
            wqk_v = w_qk8[:].rearrange("p (c m o) -> p c m o", c=2, m=2)

            def front_a(s, st):
                # ---- A: loads ---------------------------------------------
                xp8 = xpool.tile([128, 2 * NPP], FP8, tag="xp8", name="xp8")
                dma(
                    out=xp8[:].rearrange("p (c n) -> p c n", c=2),
                    in_=xs8[s].rearrange("c p n -> p c n"),
                )
                xpb = xbpool.tile([128, 2 * NPP], BF16, tag="xpb", name="xpb")
                dma(
                    out=xpb[:].rearrange("p (c n) -> p c n", c=2),
                    in_=xsb[s].rearrange("c p n -> p c n"),
                )
                xt8 = xtpool.tile([128, JT * 257], FP8, tag="xt8", name="xt8")
                dma(
                    out=xt8[:].rearrange("p (j n) -> p j n", j=JT),
                    in_=xts[s].rearrange("j p n -> p j n"),
                )
                xp8_v = xp8[:].rearrange("p (c n) -> p c n", c=2)
                st.update(xp8=xp8, xpb=xpb, xt8=xt8)

                # ---- B: q chan-major fp8 DR + elu -------------------------
                q_elu = qpool.tile([128, NPP], BF16, tag="qelu", name="q_elu")
                st.update(q=q_elu)
                for ch in range(NCH):
                    p1 = span_start(ch)
                    ps = qps.tile([128, SPAN], F32, tag="qps", name="ps")
                    nc.tensor.matmul(
                        ps[:], wqk_v[:, :, 0, :], xp8_v[:, :, p1 : p1 + SPAN],
                        start=True, stop=True, perf_mode=DR,
                    )
                    e = erpool.tile([128, SPAN], BF16, tag="e", name="e")
                    nc.scalar.activation(e[:], ps[:], Exp, bias=w_bq[:], scale=1.0)
                    r = erpool.tile([128, SPAN], BF16, tag="r", name="r")
                    if RELU_Q == "act":
                        nc.scalar.activation(r[:], ps[:], Relu, bias=w_bq[:], scale=1.0)
                    else:
                        nc.vector.tensor_scalar(
                            out=r[:], in0=ps[:], scalar1=w_bq[:], scalar2=0.0,
                            op0=ad, op1=mx,
                        )
                    m = erpool.tile([128, SPAN], BF16, tag="m", name="m")
                    nc.vector.tensor_scalar(
                        out=m[:], in0=e[:], scalar1=1.0, scalar2=None, op0=mi
                    )
                    nc.gpsimd.tensor_tensor(
                        q_elu[:, p1 : p1 + SPAN], m[:], r[:], op=ad
                    )

                # ---- C: kT pos-major fp8 DR + elu -> fp8 ------------------
                kT8 = ktpool.tile([128, JT * 128], FP8, tag="kt8", name="kT8")
                st.update(kT8=kT8)
                for g, chunks in enumerate(KT_GROUPS):
                    gs = len(chunks)
                    kp = ktps.tile([128, 128 * gs], F32, tag="ktps", name="kp")
                    nc.tensor.matmul(
                        kp[:], w_ones[:], w_kb[:, 0 : 128 * gs],
                        start=True, stop=False, skip_group_check=True,
                    )
                    for jj, j in enumerate(chunks):
                        nc.tensor.matmul(
                            kp[:, 128 * jj : 128 * (jj + 1)],
                            xp8_v[:, :, 128 * j : 128 * (j + 1)],
                            wqk_v[:, :, 1, :],
                            start=False, stop=True, perf_mode=DR,
                            skip_group_check=True,
                        )
                    e = er2pool.tile([128, 128 * gs], BF16, tag="e2", name="e2")
                    nc.scalar.activation(e[:], kp[:], Exp, bias=0.0, scale=1.0)
                    r = er2pool.tile([128, 128 * gs], BF16, tag="r2", name="r2")
                    nc.vector.tensor_scalar(
                        out=r[:], in0=kp[:], scalar1=0.0, scalar2=None, op0=mx
                    )
                    m = er2pool.tile([128, 128 * gs], BF16, tag="m2", name="m2")
                    nc.vector.tensor_scalar(
                        out=m[:], in0=e[:], scalar1=1.0, scalar2=None, op0=mi
                    )
                    with nc.allow_low_precision(reason="k in fp8: verified 4.5e-3"):
                        nc.vector.tensor_tensor(
                            kT8[:, 128 * chunks[0] : 128 * (chunks[-1] + 1)],
                            m[:], r[:], op=ad,
                        )

            def front_b(s, st):
                xt8, kT8, q_elu = st["xt8"], st["kT8"], st["q"]
                kT_v = kT8[:].rearrange("p (j c) -> p j c", j=JT)
                xt_v = xt8[:].rearrange("p (j n) -> p j n", j=JT)
                # ---- D: kv fp8 DR over chunk pairs (+ksum col 256) --------
                kvp = kvps.tile([128, 257], F32, tag="kvps", name="kvp")
                npair = JT // 2  # 13 pairs + 1 leftover chunk
                for i in range(npair):
                    nc.tensor.matmul(
                        kvp[:], kT_v[:, 2 * i : 2 * i + 2, :],
                        xt_v[:, 2 * i : 2 * i + 2, :],
                        start=(i == 0), stop=False, perf_mode=DR,
                    )
                nc.tensor.matmul(
                    kvp[:], kT_v[:, JT - 1, :], xt_v[:, JT - 1, :],
                    start=False, stop=True,
                )
                kv_sb = kvpool.tile([128, 257], BF16, tag="kv", name="kv_sb")
                nc.vector.tensor_copy(kv_sb[:], kvp[:])
                st.update(kv=kv_sb)

                # ---- E: den + reciprocal (DVE, straight from psum) --------
                rrow = denpool.tile([1, NP], BF16, tag="rrow", name="rrow")
                for ch in range(NCH):
                    p1 = span_start(ch)
                    dp = dbps.tile([1, SPAN], F32, tag="dbps", name="dp")
                    nc.tensor.matmul(
                        dp[:], kv_sb[:, 256:257], q_elu[:, p1 : p1 + SPAN],
                        start=True, stop=True,
                    )
                    with nc.allow_low_precision(reason="bf16 recip row: 4e-3 rel ok"):
                        nc.vector.reciprocal(rrow[:, p1 : p1 + SPAN], dp[:])
                st.update(rrow=rrow)

            def back(s, st):
                xpb, q_elu, kv_sb, rrow = st["xpb"], st["q"], st["kv"], st["rrow"]
                # ---- F1: recip broadcast + q scaling ----------------------
                for ch in range(NCH):
                    p1 = span_start(ch)
                    bc = bcps.tile([128, SPAN], F32, tag="bcps", name="bc")
                    nc.tensor.matmul(
                        bc[:], w_ones[:], rrow[:, p1 : p1 + SPAN],
                        start=True, stop=True,
                    )
                    nc.vector.tensor_tensor(
                        q_elu[:, p1 : p1 + SPAN], q_elu[:, p1 : p1 + SPAN],
                        bc[:], op=mu,
                    )

                # ---- F2: vector conv chains for offloaded chunks ----------
                accs = {}
                for cb in range(2):
                    for ch in range(NCH):
                        eng = ("dve" if (cb, ch) in VEC_DVE
                               else "pool" if (cb, ch) in VEC_POOL else None)
                        if eng is None:
                            continue
                        p1 = span_start(ch)
                        base = NPP * cb + p1
                        acc = accpool.tile([128, SPAN], BF16, tag="acc", name="acc")
                        accs[(cb, ch)] = acc
                        if eng == "dve":
                            nc.vector.tensor_scalar(
                                out=acc[:], in0=xpb[:, base : base + SPAN],
                                scalar1=w_tapv[:, 9 * cb + 4 : 9 * cb + 5],
                                scalar2=None, op0=mu,
                            )
                            for t in range(9):
                                if t == 4:
                                    continue
                                off = HP * (t // 3 - 1) + (t % 3 - 1)
                                tmp = accpool.tile(
                                    [128, SPAN], BF16, tag="tmp", name="tmp"
                                )
                                nc.vector.tensor_scalar(
                                    out=tmp[:], in0=xpb[:, base + off : base + off + SPAN],
                                    scalar1=w_tapv[:, 9 * cb + t : 9 * cb + t + 1],
                                    scalar2=None, op0=mu,
                                )
                                nc.vector.tensor_tensor(acc[:], acc[:], tmp[:], op=ad)
                        else:
                            nc.gpsimd.tensor_scalar(
                                out=acc[:], in0=xpb[:, base : base + SPAN],
                                scalar1=w_tapv[:, 9 * cb + 4 : 9 * cb + 5],
                                scalar2=None, op0=mu,
                            )
                            for t in range(9):
                                if t == 4:
                                    continue
                                off = HP * (t // 3 - 1) + (t % 3 - 1)
                                nc.gpsimd.scalar_tensor_tensor(
                                    acc[:], xpb[:, base + off : base + off + SPAN],
                                    w_tapv[:, 9 * cb + t : 9 * cb + t + 1], acc[:],
                                    op0=mu, op1=ad,
                                )

                # ---- F3: num (+PE taps) + evacuation ----------------------
                for cb in range(2):
                    o16 = opool.tile([128, N], F16, tag="o16", name="o16")
                    for ch in range(NCH):
                        p1 = span_start(ch)
                        is_vec = (cb, ch) in accs
                        pn = numps.tile([128, SPAN], F32, tag="numps", name="pn")
                        nc.tensor.matmul(
                            pn[:], kv_sb[:, 128 * cb : 128 * (cb + 1)],
                            q_elu[:, p1 : p1 + SPAN],
                            start=True, stop=is_vec,
                        )
                        if not is_vec:
                            for t in range(9):
                                off = HP * (t // 3 - 1) + (t % 3 - 1)
                                nc.tensor.matmul(
                                    pn[:],
                                    w_tap[:, 1152 * cb + 128 * t : 1152 * cb + 128 * (t + 1)],
                                    xpb[:, NPP * cb + p1 + off : NPP * cb + p1 + off + SPAN],
                                    start=False, stop=(t == 8),
                                )
                        o_v = o16[:, 448 * ch : 448 * (ch + 1)].rearrange(
                            "p (y x) -> p y x", x=56
                        )
                        pn_v = pn[:].rearrange("p (y x) -> p y x", x=HP)[:, :, 0:56]
                        with nc.allow_low_precision(reason="fp16 out: verified 4.5e-3"):
                            if is_vec:
                                acc_v = accs[(cb, ch)][:].rearrange(
                                    "p (y x) -> p y x", x=HP
                                )[:, :, 0:56]
                                nc.vector.scalar_tensor_tensor(
                                    o_v, pn_v, w_peb[:, cb : cb + 1], acc_v,
                                    op0=ad, op1=ad,
                                )
                            else:
                                nc.scalar.activation(
                                    o_v, pn_v,
                                    mybir.ActivationFunctionType.Identity,
                                    bias=w_peb[:, cb : cb + 1], scale=1.0,
                                )
                    dma(out=out[s, cb], in_=o16[:])

            states = [dict() for _ in range(SPC)]
            for s in range(SPC):
                front_a(s, states[s])
                if s >= 1:
                    back(s - 1, states[s - 1])
                front_b(s, states[s])
            back(SPC - 1, states[SPC - 1])

    _hoist_waits(nc, (mybir.InstDmaTransposeAnt,), 0)
    _hoist_waits(nc, None, 1)
    return nc


_NC_CACHE = []


def _host_prep(x, qk_w, qk_b, pe_w, pe_b):
    bf16 = ml_dtypes.bfloat16
    fp8 = ml_dtypes.float8_e4m3
    # padded 58x58 spatial layout, tail-padded to 3456; c in two 128-blocks
    xp = np.zeros((B, 2, 128, HP, HP), np.float32)
    xp[:, :, :, 1 : H + 1, 1 : W + 1] = x.reshape(B, 2, 128, H, W)
    xpf = np.zeros((B, 2, 128, NPP), np.float32)
    xpf[:, :, :, :NP] = xp.reshape(B, 2, 128, NP)
    xs8 = xpf.astype(fp8)
    xsb = xpf.astype(bf16)

    # transposed layout with interior-masked ones column
    xT = xpf.reshape(B, 256, NPP).transpose(0, 2, 1)  # [B, NPP, 256]
    pos = np.arange(NPP)
    interior = ((pos < NP) & ((pos // HP) >= 1) & ((pos // HP) <= H)
                & ((pos % HP) >= 1) & ((pos % HP) <= W))
    xts = np.concatenate(
        [xT, np.broadcast_to(interior[None, :, None], (B, NPP, 1)).astype(np.float32)],
        axis=2,
    )  # [B, NPP, 257]
    xts = np.ascontiguousarray(xts.reshape(B, JT, 128, 257)).astype(fp8)

    # packed DR qk weight: [p=c_in%128, cc=c_in//128, mb(q/k), o]
    wqk8 = np.zeros((128, 2, 2, 128), np.float32)
    for cc in range(2):
        for mb in range(2):
            wqk8[:, cc, mb, :] = qk_w[128 * mb : 128 * (mb + 1),
                                      128 * cc : 128 * (cc + 1)].T
    wqk8 = wqk8.astype(fp8)
    wkb = np.tile(qk_b[128:256], 4).astype(bf16)  # k bias row, repeated per chunk

    wtap = np.zeros((2, 9, 128, 128), np.float32)
    idx = np.arange(128)
    for cb in range(2):
        for t in range(9):
            wtap[cb, t, idx, idx] = pe_w[128 * cb : 128 * (cb + 1), 0, t // 3, t % 3]
    wtap = wtap.astype(bf16)
    wtapv = np.zeros((128, 18), np.float32)
    for cb in range(2):
        for t in range(9):
            wtapv[:, 9 * cb + t] = pe_w[128 * cb : 128 * (cb + 1), 0, t // 3, t % 3]
    pebh = np.stack([pe_b[:128], pe_b[128:]], axis=1).copy()

    shared = {
        "wqk8": wqk8.ravel(),
        "wkb": wkb.ravel(),
        "wtap": wtap.ravel(),
        "wtapv": wtapv.ravel(),
        "onesb": np.ones(128, bf16),
        "biasq": np.ascontiguousarray(qk_b[:128]),
        "peb": pebh.ravel(),
    }
    return xs8, xsb, xts, shared


def kernel(x, qk_w, qk_b, pe_w, pe_b):
    x = np.asarray(x, np.float32)
    qk_w = np.asarray(qk_w, np.float32)
    qk_b = np.asarray(qk_b, np.float32)
    pe_w = np.asarray(pe_w, np.float32)
    pe_b = np.asarray(pe_b, np.float32)

    xs8, xsb, xts, shared = _host_prep(x, qk_w, qk_b, pe_w, pe_b)
    in_maps = [
        {
            "xs8": np.ascontiguousarray(xs8[r * SPC : (r + 1) * SPC]).ravel(),
            "xsb": np.ascontiguousarray(xsb[r * SPC : (r + 1) * SPC]).ravel(),
            "xts": np.ascontiguousarray(xts[r * SPC : (r + 1) * SPC]).ravel(),
            **shared,
        }
        for r in range(NCORES)
    ]

    if not _NC_CACHE:
        _NC_CACHE.append(_build())
    nc = _NC_CACHE[0]
    res = run_bass_kernel_spmd(nc, in_maps, list(range(NCORES)))

    full = np.empty((B, C, H, W), np.float32)
    for r in range(NCORES):
        o = res.results[r]["out"].reshape(SPC, 2, 128, N).astype(np.float32)
        full[r * SPC : (r + 1) * SPC] = o.reshape(SPC, C, H, W)
    return full
